# revision 14
# baseline (speedup 1.0000x reference)
"""Trainium2 Bass kernel for nn_ConsistencyLoss (BCE + dilated-stencil consistency loss).

loss = mean( unfolded_weights * thred + bce )
  bce      = -(y_true*max(log(y_pred),-100) + (1-y_true)*max(log1p(-y_pred),-100))
  unfolded = max over 8 dilated (DIL=2) neighbors nb of |y_pred - nb|, zero-padded
  thred    = y_pred * (y_pred >= 0.5)

Strategy (8 NeuronCores, data-parallel over batch, 2 images/core):
  - Chunk tiles [128, 4096] = 2 bands x 2 images, blocks [i0b0|i0b1|i1b0|i1b1].
  - unfolded = max(c - nmin, nmax - c); nmax/nmin separable over the dilated
    3x3 window INCLUDING the center (including the center never changes the
    result since |c-c| = 0 <= unfolded).
  - Vertical (partition) shifts via SBUF->SBUF DMA; horizontal shifts via
    free-dim slices of zero-padded tiles. Stencil in bf16 on DVE (2x mode).
  - BCE logs on ScalarE: ln(x + FLT_MIN) reproduces torch's -100 clamp for
    uniform inputs (only x == 0 clamps; contribution ~1e-6 relative).
  - Product-sums (U*R, U*m, yt*lp, yt*l1p) via TensorE diagonal matmuls
    accumulated in PSUM, rhs blocks interleaved 128-col for contiguity;
    sum(l1p) rides the ACT pass accum_out. Host assembles the scalar.
"""

from contextlib import ExitStack

import numpy as np

import concourse.bacc as bacc
import concourse.tile as tile
from concourse import mybir
from concourse.bass_utils import run_bass_kernel_spmd

F32 = mybir.dt.float32
BF16 = mybir.dt.bfloat16
OP = mybir.AluOpType
AT = mybir.ActivationFunctionType

B, H, W = 16, 1024, 1024
NCORES = 8
IPC = B // NCORES          # images per core = 2
P = 128
NB = 2                     # bands per image per chunk tile
NBLK = IPC * NB            # 4 column blocks per chunk tile
NCHUNK = H // (P * NB)     # 4 chunk iterations
FW = NBLK * W              # 4096
BW = W + 4                 # padded block width
DIL = 2
TINY = 1.18e-38            # min normal fp32; ln(x+TINY) == ln(x) for x >= 2^-24

NACC = 4                   # round-robin PSUM accumulators per stream
N_OUT = 2 * NACC * 256 + NCHUNK


def blkc(q):
    """column range of block q"""
    return q * W, (q + 1) * W


def _kernel_body(ctx, tc, yp, yt, out):
    nc = tc.nc

    xpool = ctx.enter_context(tc.tile_pool(name="xpool", bufs=2))
    xbpool = ctx.enter_context(tc.tile_pool(name="xbpool", bufs=3))
    ytpool = ctx.enter_context(tc.tile_pool(name="ytpool", bufs=1))
    fpool = ctx.enter_context(tc.tile_pool(name="fpool", bufs=2))    # lpl1p / rm
    shpool = ctx.enter_context(tc.tile_pool(name="shpool", bufs=1))  # xu/xd
    vpool = ctx.enter_context(tc.tile_pool(name="vpool", bufs=1))    # padded vmax/vmin
    spool = ctx.enter_context(tc.tile_pool(name="spool", bufs=1))    # stencil temps
    upool = ctx.enter_context(tc.tile_pool(name="upool", bufs=1))
    single = ctx.enter_context(tc.tile_pool(name="single", bufs=1))
    psum = ctx.enter_context(tc.tile_pool(name="psum", bufs=1, space="PSUM"))

    l1pacc = single.tile([P, NCHUNK], F32)
    psum_a = [psum.tile([P, 256], F32, name=f"psum_a{k}") for k in range(NACC)]
    psum_b = [psum.tile([P, 256], F32, name=f"psum_b{k}") for k in range(NACC)]

    bias_tiny = single.tile([P, 1], F32)
    nc.gpsimd.memset(bias_tiny, TINY)
    bias_one = single.tile([P, 1], F32)
    nc.gpsimd.memset(bias_one, 1.0)
    bias_neghalf = single.tile([P, 1], F32)
    nc.gpsimd.memset(bias_neghalf, -0.5)

    zrow = single.tile([DIL, W], BF16)
    nc.vector.memset(zrow, 0.0)

    xb_tiles = {}
    rm_tiles = {}

    n_pieces = FW // P  # 32 lhsT pieces per chunk per stream

    def band_rows(c, s):
        r0 = (c * NB + s) * P
        return r0, r0 + P

    def load_chunk(c):
        x = xpool.tile([P, FW], F32, name=f"x_{c}", tag="x")
        xb = xbpool.tile([P, FW], BF16, name=f"xb_{c}", tag="xb")
        ytb = ytpool.tile([P, FW], BF16, name=f"ytb_{c}", tag="ytb")
        for img in range(IPC):
            for s in range(NB):
                q = img * NB + s
                c0, c1 = blkc(q)
                r0, r1 = band_rows(c, s)
                nc.sync.dma_start(out=x[:, c0:c1], in_=yp[img, r0:r1, :])
                # casting loads (SWDGE)
                nc.gpsimd.dma_start(out=xb[:, c0:c1], in_=yp[img, r0:r1, :])
                nc.gpsimd.dma_start(out=ytb[:, c0:c1], in_=yt[img, r0:r1, :])
        xb_tiles[c] = xb

        # [lp|l1p] interleaved at 128 cols: piece j occupies cols [256j, 256j+256)
        lpl1p = fpool.tile([P, 2 * FW], BF16, name=f"lpl1p_{c}", tag="lpl1p")
        lp4 = lpl1p.rearrange("p (j t w) -> p j t w", t=2, w=P)
        nc.scalar.activation(lp4[:, :, 0, :], x, AT.Ln, bias=bias_tiny, scale=1.0)
        nc.scalar.activation(
            lp4[:, :, 1, :], x, AT.Ln, bias=bias_one, scale=-1.0,
            accum_out=l1pacc[:, c:c + 1],
        )

        # [R|m] interleaved the same way; R on ACT, m on DVE
        rm = fpool.tile([P, 2 * FW], BF16, name=f"rm_{c}", tag="rm")
        rm4 = rm.rearrange("p (j t w) -> p j t w", t=2, w=P)
        nc.scalar.activation(rm4[:, :, 0, :], x, AT.Relu, bias=bias_neghalf, scale=1.0)
        nc.vector.tensor_scalar(
            out=rm4[:, :, 1, :], in0=x, scalar1=0.5, scalar2=None, op0=OP.is_ge,
        )
        rm_tiles[c] = rm

        # BCE product-sums: psum_b[m, :] += sum_k ytb[k, 128j+m] * [lp|l1p](j)[k, :]
        for j in range(n_pieces):
            nc.tensor.matmul(
                psum_b[j % NACC],
                ytb[:, j * P:(j + 1) * P],
                lpl1p[:, j * 256:(j + 1) * 256],
                start=(c == 0 and j < NACC),
                stop=(c == NCHUNK - 1 and j >= n_pieces - NACC),
            )

    def stencil_chunk(c):
        xbc = xb_tiles[c]

        # vertical +-2 partition shifts; per-block halo fixups
        xu = shpool.tile([P, FW], BF16, name=f"xu_{c}", tag="xu")
        xd = shpool.tile([P, FW], BF16, name=f"xd_{c}", tag="xd")
        nc.sync.dma_start(out=xu[0:P - DIL, :], in_=xbc[DIL:P, :])
        nc.sync.dma_start(out=xd[DIL:P, :], in_=xbc[0:P - DIL, :])
        for img in range(IPC):
            for s in range(NB):
                q = img * NB + s
                c0, c1 = blkc(q)
                # bottom halo of block q: first rows of the next band down
                if s + 1 < NB:
                    n0, n1 = blkc(img * NB + s + 1)
                    nc.sync.dma_start(out=xu[P - DIL:P, c0:c1], in_=xbc[0:DIL, n0:n1])
                elif c + 1 < NCHUNK:
                    n0, n1 = blkc(img * NB)
                    nc.sync.dma_start(
                        out=xu[P - DIL:P, c0:c1], in_=xb_tiles[c + 1][0:DIL, n0:n1])
                else:
                    nc.sync.dma_start(out=xu[P - DIL:P, c0:c1], in_=zrow)
                # top halo of block q: last rows of the previous band up
                if s > 0:
                    n0, n1 = blkc(img * NB + s - 1)
                    nc.sync.dma_start(out=xd[0:DIL, c0:c1], in_=xbc[P - DIL:P, n0:n1])
                elif c > 0:
                    n0, n1 = blkc(img * NB + NB - 1)
                    nc.sync.dma_start(
                        out=xd[0:DIL, c0:c1], in_=xb_tiles[c - 1][P - DIL:P, n0:n1])
                else:
                    nc.sync.dma_start(out=xd[0:DIL, c0:c1], in_=zrow)

        # vertical 3-max / 3-min into zero-padded tiles
        vmax = vpool.tile([P, NBLK * BW], BF16, name=f"vmax_{c}", tag="vmax")
        vmin = vpool.tile([P, NBLK * BW], BF16, name=f"vmin_{c}", tag="vmin")
        for v in (vmax, vmin):
            for q in range(NBLK):
                nc.gpsimd.memset(v[:, q * BW:q * BW + 2], 0.0)
                nc.gpsimd.memset(v[:, q * BW + BW - 2:(q + 1) * BW], 0.0)
        vmax3 = vmax.rearrange("p (q w) -> p q w", q=NBLK)
        vmin3 = vmin.rearrange("p (q w) -> p q w", q=NBLK)

        def b3(t):
            return t.rearrange("p (q w) -> p q w", q=NBLK)

        va = spool.tile([P, FW], BF16, name=f"va_{c}", tag="g1")
        nc.vector.tensor_tensor(out=va, in0=xu, in1=xd, op=OP.max)
        nc.vector.tensor_tensor(
            out=vmax3[:, :, 2:2 + W], in0=b3(va), in1=b3(xbc), op=OP.max)
        vb = spool.tile([P, FW], BF16, name=f"vb_{c}", tag="g2")
        nc.vector.tensor_tensor(out=vb, in0=xu, in1=xd, op=OP.min)
        nc.vector.tensor_tensor(
            out=vmin3[:, :, 2:2 + W], in0=b3(vb), in1=b3(xbc), op=OP.min)

        # horizontal dilated 3-max / 3-min
        nxa = spool.tile([P, FW], BF16, name=f"nxa_{c}", tag="g1")
        nc.vector.tensor_tensor(
            out=b3(nxa), in0=vmax3[:, :, 0:W], in1=vmax3[:, :, 4:4 + W], op=OP.max)
        nx = spool.tile([P, FW], BF16, name=f"nx_{c}", tag="g3")
        nc.vector.tensor_tensor(
            out=b3(nx), in0=b3(nxa), in1=vmax3[:, :, 2:2 + W], op=OP.max)
        nma = spool.tile([P, FW], BF16, name=f"nma_{c}", tag="g2")
        nc.vector.tensor_tensor(
            out=b3(nma), in0=vmin3[:, :, 0:W], in1=vmin3[:, :, 4:4 + W], op=OP.min)
        nm = spool.tile([P, FW], BF16, name=f"nm_{c}", tag="g4")
        nc.vector.tensor_tensor(
            out=b3(nm), in0=b3(nma), in1=vmin3[:, :, 2:2 + W], op=OP.min)

        # unfolded = max(xb - nmin, nmax - xb)
        u1 = spool.tile([P, FW], BF16, name=f"u1_{c}", tag="g1")
        nc.vector.tensor_tensor(out=u1, in0=xbc, in1=nm, op=OP.subtract)
        u2 = spool.tile([P, FW], BF16, name=f"u2_{c}", tag="g2")
        nc.vector.tensor_tensor(out=u2, in0=nx, in1=xbc, op=OP.subtract)
        u = upool.tile([P, FW], BF16, name=f"u_{c}", tag="u")
        nc.vector.tensor_tensor(out=u, in0=u1, in1=u2, op=OP.max)

        # psum_a[m, :] += sum_k u[k, 128j+m] * [R|m](j)[k, :]
        rmc = rm_tiles[c]
        for j in range(n_pieces):
            nc.tensor.matmul(
                psum_a[j % NACC],
                u[:, j * P:(j + 1) * P],
                rmc[:, j * 256:(j + 1) * 256],
                start=(c == 0 and j < NACC),
                stop=(c == NCHUNK - 1 and j >= n_pieces - NACC),
            )

    # software pipeline: load chunk c while running the stencil on chunk c-1
    for c in range(NCHUNK + 1):
        if c < NCHUNK:
            load_chunk(c)
        if c >= 1:
            stencil_chunk(c - 1)

    for k in range(2 * NACC):
        src = psum_a[k] if k < NACC else psum_b[k - NACC]
        res = single.tile([P, 256], F32, name=f"res_{k}", tag="res", bufs=2)
        nc.vector.tensor_copy(out=res, in_=src)
        nc.sync.dma_start(out=out[:, k * 256:(k + 1) * 256], in_=res)
    nc.sync.dma_start(out=out[:, 2 * NACC * 256:N_OUT], in_=l1pacc)


_CACHED = {}


def _build():
    if "nc" in _CACHED:
        return _CACHED["nc"]
    nc = bacc.Bacc(
        "TRN2",
        target_bir_lowering=False,
        debug=False,
        num_devices=NCORES,
    )
    yp = nc.dram_tensor("y_pred", [IPC, H, W], F32, kind="ExternalInput").ap()
    yt = nc.dram_tensor("y_true", [IPC, H, W], F32, kind="ExternalInput").ap()
    out = nc.dram_tensor("out", [P, N_OUT], F32, kind="ExternalOutput").ap()
    with tile.TileContext(nc) as tc:
        with ExitStack() as ctx:
            _kernel_body(ctx, tc, yp, yt, out)
    nc.compile()
    _CACHED["nc"] = nc
    return nc


def _host_reduce(outs):
    """Assemble the scalar loss from the 8 per-core [P, N_OUT] partial tensors."""
    total = np.float64(0.0)
    idx = np.arange(P)
    for o in outs:
        o = np.asarray(o, dtype=np.float64)
        a = o[:, 0:NACC * 256].reshape(P, NACC, 256).sum(axis=1)
        bq = o[:, NACC * 256:2 * NACC * 256].reshape(P, NACC, 256).sum(axis=1)
        l1 = o[:, 2 * NACC * 256:2 * NACC * 256 + NCHUNK]
        sum_ur = a[idx, idx].sum()          # sum U * relu(x-.5)
        sum_um = a[idx, 128 + idx].sum()    # sum U * (x>=.5)
        sum_ylp = bq[idx, idx].sum()        # sum yt * ln(x)
        sum_yl1p = bq[idx, 128 + idx].sum() # sum yt * ln(1-x)
        sum_l1p = l1.sum()                  # sum ln(1-x)
        total += (sum_ur + 0.5 * sum_um) - sum_ylp - sum_l1p + sum_yl1p
    return np.float32(total / (B * H * W))


def kernel(y_true, y_pred):
    y_true = np.ascontiguousarray(np.asarray(y_true, dtype=np.float32)).reshape(B, H, W)
    y_pred = np.ascontiguousarray(np.asarray(y_pred, dtype=np.float32)).reshape(B, H, W)

    nc = _build()
    in_maps = []
    for r in range(NCORES):
        in_maps.append({
            "y_pred": np.ascontiguousarray(y_pred[r * IPC:(r + 1) * IPC]),
            "y_true": np.ascontiguousarray(y_true[r * IPC:(r + 1) * IPC]),
        })
    res = run_bass_kernel_spmd(nc, in_maps, core_ids=list(range(NCORES)))
    outs = [res.results[r]["out"] for r in range(NCORES)]
    return _host_reduce(outs)


# revision 17
# speedup vs baseline: 1.0253x; 1.0253x over previous
"""Trainium2 Bass kernel for nn_ConsistencyLoss (BCE + dilated-stencil consistency loss).

loss = mean( unfolded_weights * thred + bce )
  bce      = -(y_true*max(log(y_pred),-100) + (1-y_true)*max(log1p(-y_pred),-100))
  unfolded = max over 8 dilated (DIL=2) neighbors nb of |y_pred - nb|, zero-padded
  thred    = y_pred * (y_pred >= 0.5)

Strategy (8 NeuronCores, data-parallel over batch, 2 images/core):
  - Chunk tiles [128, 4096] = 2 bands x 2 images, blocks [i0b0|i0b1|i1b0|i1b1];
    one 4D-AP DMA per tensor per chunk (casting loads for bf16 copies).
  - unfolded = max(c - nmin, nmax - c); nmax/nmin separable over the dilated
    3x3 window INCLUDING the center (|c-c| = 0 never changes the max).
  - Vertical (partition) shifts via SBUF->SBUF DMA; horizontal shifts via
    free-dim slices of zero-padded tiles. Stencil in bf16 on DVE (2x mode).
  - BCE logs + relu(x-.5) + sign(x-.5) on ScalarE: ln(x + FLT_MIN)
    reproduces torch's -100 clamp for uniform inputs (only x == 0 clamps).
    thred = R + 0.25*s + 0.25 with R = relu(x-.5), s = sign(x-.5).
  - Product-sums via TensorE diagonal matmuls accumulated in PSUM:
    a-stream rhs pieces [R_j | s_j | 1] (FD=257, the ones column yields
    sum(U) for free), b-stream [lp_j | l1p_j]; 4 round-robin accumulators
    per stream; sum(l1p) rides the ACT accum_out. Host assembles the scalar.
"""

from contextlib import ExitStack

import numpy as np

import concourse.bacc as bacc
import concourse.tile as tile
from concourse import mybir
from concourse.bass_utils import run_bass_kernel_spmd

F32 = mybir.dt.float32
BF16 = mybir.dt.bfloat16
OP = mybir.AluOpType
AT = mybir.ActivationFunctionType

B, H, W = 16, 1024, 1024
NCORES = 8
IPC = B // NCORES          # images per core = 2
P = 128
NB = 2                     # bands per image per chunk tile
NBLK = IPC * NB            # 4 column blocks per chunk tile
NCHUNK = H // (P * NB)     # 4 chunk iterations
FW = NBLK * W              # 4096
BW = W + 4                 # padded block width
DIL = 2
TINY = 1.18e-38            # min normal fp32; ln(x+TINY) == ln(x) for x >= 2^-24

NACC = 4                   # round-robin PSUM accumulators per stream
RSTR = 260                 # rhs piece stride (els) in the [R|s|1] tile (8B-aligned)
AW = 257                   # a-stream rhs width: [R(128) | s(128) | ones(1)]
N_OUT = NACC * AW + NACC * 256 + NCHUNK


def _kernel_body(ctx, tc, yp, yt, out):
    nc = tc.nc

    xpool = ctx.enter_context(tc.tile_pool(name="xpool", bufs=2))
    xbpool = ctx.enter_context(tc.tile_pool(name="xbpool", bufs=3))
    ytpool = ctx.enter_context(tc.tile_pool(name="ytpool", bufs=1))
    fpool = ctx.enter_context(tc.tile_pool(name="fpool", bufs=2))    # lpl1p / rs1
    shpool = ctx.enter_context(tc.tile_pool(name="shpool", bufs=1))  # xu/xd
    vpool = ctx.enter_context(tc.tile_pool(name="vpool", bufs=1))    # padded vmax/vmin
    spool = ctx.enter_context(tc.tile_pool(name="spool", bufs=1))    # stencil temps
    upool = ctx.enter_context(tc.tile_pool(name="upool", bufs=1))
    single = ctx.enter_context(tc.tile_pool(name="single", bufs=1))
    psum = ctx.enter_context(tc.tile_pool(name="psum", bufs=1, space="PSUM"))

    l1pacc = single.tile([P, NCHUNK], F32)
    psum_a = [psum.tile([P, AW], F32, name=f"psum_a{k}") for k in range(NACC)]
    psum_b = [psum.tile([P, 256], F32, name=f"psum_b{k}") for k in range(NACC)]

    bias_tiny = single.tile([P, 1], F32)
    nc.gpsimd.memset(bias_tiny, TINY)
    bias_one = single.tile([P, 1], F32)
    nc.gpsimd.memset(bias_one, 1.0)
    bias_neghalf = single.tile([P, 1], F32)
    nc.gpsimd.memset(bias_neghalf, -0.5)

    zrow = single.tile([DIL, W], BF16)
    nc.vector.memset(zrow, 0.0)

    xb_tiles = {}
    rs_tiles = {}

    n_pieces = FW // P  # 32 lhsT pieces per chunk per stream

    def chunk_src(t, c, img):
        """[NB*P, W] DRAM rows of chunk c, image img -> [P, band, w] 3D AP."""
        return t[img, c * NB * P:(c + 1) * NB * P, :].rearrange(
            "(s p) w -> p s w", p=P)

    def load_chunk(c):
        x = xpool.tile([P, FW], F32, name=f"x_{c}", tag="x")
        xb = xbpool.tile([P, FW], BF16, name=f"xb_{c}", tag="xb")
        ytb = ytpool.tile([P, FW], BF16, name=f"ytb_{c}", tag="ytb")
        for img in range(IPC):
            h0 = img * NB * W
            o3 = lambda t: t[:, h0:h0 + NB * W].rearrange("p (s w) -> p s w", s=NB)
            nc.sync.dma_start(out=o3(x), in_=chunk_src(yp, c, img))
            nc.gpsimd.dma_start(out=o3(xb), in_=chunk_src(yp, c, img))
            nc.gpsimd.dma_start(out=o3(ytb), in_=chunk_src(yt, c, img))
        xb_tiles[c] = xb

        # [lp|l1p] interleaved at 128 cols: piece j occupies cols [256j, 256j+256)
        lpl1p = fpool.tile([P, 2 * FW], BF16, name=f"lpl1p_{c}", tag="lpl1p")
        lp4 = lpl1p.rearrange("p (j t w) -> p j t w", t=2, w=P)
        nc.scalar.activation(lp4[:, :, 0, :], x, AT.Ln, bias=bias_tiny, scale=1.0)
        nc.scalar.activation(
            lp4[:, :, 1, :], x, AT.Ln, bias=bias_one, scale=-1.0,
            accum_out=l1pacc[:, c:c + 1],
        )

        # [R|s|1] pieces with stride RSTR; R, s on ACT; ones via memset
        rs1 = fpool.tile([P, n_pieces * RSTR], BF16, name=f"rs1_{c}", tag="rs1")
        rs4 = rs1.rearrange("p (j w) -> p j w", j=n_pieces)
        nc.scalar.activation(rs4[:, :, 0:P], x, AT.Relu, bias=bias_neghalf, scale=1.0)
        nc.scalar.activation(rs4[:, :, P:2 * P], x, AT.Sign, bias=bias_neghalf, scale=1.0)
        nc.gpsimd.memset(rs4[:, :, 2 * P:2 * P + 1], 1.0)
        rs_tiles[c] = rs1

        # BCE product-sums: psum_b[m, :] += sum_k ytb[k, 128j+m] * [lp|l1p](j)[k, :]
        for j in range(n_pieces):
            nc.tensor.matmul(
                psum_b[j % NACC],
                ytb[:, j * P:(j + 1) * P],
                lpl1p[:, j * 256:(j + 1) * 256],
                start=(c == 0 and j < NACC),
                stop=(c == NCHUNK - 1 and j >= n_pieces - NACC),
            )

    def stencil_chunk(c):
        xbc = xb_tiles[c]

        # vertical +-2 partition shifts; per-block halo fixups
        xu = shpool.tile([P, FW], BF16, name=f"xu_{c}", tag="xu")
        xd = shpool.tile([P, FW], BF16, name=f"xd_{c}", tag="xd")
        nc.sync.dma_start(out=xu[0:P - DIL, :], in_=xbc[DIL:P, :])
        nc.sync.dma_start(out=xd[DIL:P, :], in_=xbc[0:P - DIL, :])
        for img in range(IPC):
            for s in range(NB):
                q = img * NB + s
                c0, c1 = q * W, (q + 1) * W
                # bottom halo of block q: first rows of the next band down
                if s + 1 < NB:
                    n0 = (img * NB + s + 1) * W
                    nc.sync.dma_start(
                        out=xu[P - DIL:P, c0:c1], in_=xbc[0:DIL, n0:n0 + W])
                elif c + 1 < NCHUNK:
                    n0 = (img * NB) * W
                    nc.sync.dma_start(
                        out=xu[P - DIL:P, c0:c1],
                        in_=xb_tiles[c + 1][0:DIL, n0:n0 + W])
                else:
                    nc.sync.dma_start(out=xu[P - DIL:P, c0:c1], in_=zrow)
                # top halo of block q: last rows of the previous band up
                if s > 0:
                    n0 = (img * NB + s - 1) * W
                    nc.sync.dma_start(
                        out=xd[0:DIL, c0:c1], in_=xbc[P - DIL:P, n0:n0 + W])
                elif c > 0:
                    n0 = (img * NB + NB - 1) * W
                    nc.sync.dma_start(
                        out=xd[0:DIL, c0:c1],
                        in_=xb_tiles[c - 1][P - DIL:P, n0:n0 + W])
                else:
                    nc.sync.dma_start(out=xd[0:DIL, c0:c1], in_=zrow)

        # vertical 3-max / 3-min into zero-padded tiles
        vmax = vpool.tile([P, NBLK * BW], BF16, name=f"vmax_{c}", tag="vmax")
        vmin = vpool.tile([P, NBLK * BW], BF16, name=f"vmin_{c}", tag="vmin")
        for v in (vmax, vmin):
            for q in range(NBLK):
                nc.gpsimd.memset(v[:, q * BW:q * BW + 2], 0.0)
                nc.gpsimd.memset(v[:, q * BW + BW - 2:(q + 1) * BW], 0.0)
        vmax3 = vmax.rearrange("p (q w) -> p q w", q=NBLK)
        vmin3 = vmin.rearrange("p (q w) -> p q w", q=NBLK)

        def b3(t):
            return t.rearrange("p (q w) -> p q w", q=NBLK)

        va = spool.tile([P, FW], BF16, name=f"va_{c}", tag="g1")
        nc.vector.tensor_tensor(out=va, in0=xu, in1=xd, op=OP.max)
        nc.vector.tensor_tensor(
            out=vmax3[:, :, 2:2 + W], in0=b3(va), in1=b3(xbc), op=OP.max)
        vb = spool.tile([P, FW], BF16, name=f"vb_{c}", tag="g2")
        nc.vector.tensor_tensor(out=vb, in0=xu, in1=xd, op=OP.min)
        nc.vector.tensor_tensor(
            out=vmin3[:, :, 2:2 + W], in0=b3(vb), in1=b3(xbc), op=OP.min)

        # horizontal dilated 3-max / 3-min
        nxa = spool.tile([P, FW], BF16, name=f"nxa_{c}", tag="g1")
        nc.vector.tensor_tensor(
            out=b3(nxa), in0=vmax3[:, :, 0:W], in1=vmax3[:, :, 4:4 + W], op=OP.max)
        nx = spool.tile([P, FW], BF16, name=f"nx_{c}", tag="g3")
        nc.vector.tensor_tensor(
            out=b3(nx), in0=b3(nxa), in1=vmax3[:, :, 2:2 + W], op=OP.max)
        nma = spool.tile([P, FW], BF16, name=f"nma_{c}", tag="g2")
        nc.vector.tensor_tensor(
            out=b3(nma), in0=vmin3[:, :, 0:W], in1=vmin3[:, :, 4:4 + W], op=OP.min)
        nm = spool.tile([P, FW], BF16, name=f"nm_{c}", tag="g4")
        nc.vector.tensor_tensor(
            out=b3(nm), in0=b3(nma), in1=vmin3[:, :, 2:2 + W], op=OP.min)

        # unfolded = max(xb - nmin, nmax - xb)
        u1 = spool.tile([P, FW], BF16, name=f"u1_{c}", tag="g1")
        nc.vector.tensor_tensor(out=u1, in0=xbc, in1=nm, op=OP.subtract)
        u2 = spool.tile([P, FW], BF16, name=f"u2_{c}", tag="g2")
        nc.vector.tensor_tensor(out=u2, in0=nx, in1=xbc, op=OP.subtract)
        u = upool.tile([P, FW], BF16, name=f"u_{c}", tag="u")
        nc.vector.tensor_tensor(out=u, in0=u1, in1=u2, op=OP.max)

        # psum_a[m, :] += sum_k u[k, 128j+m] * [R|s|1](j)[k, :]
        rsc = rs_tiles[c]
        for j in range(n_pieces):
            nc.tensor.matmul(
                psum_a[j % NACC],
                u[:, j * P:(j + 1) * P],
                rsc[:, j * RSTR:j * RSTR + AW],
                start=(c == 0 and j < NACC),
                stop=(c == NCHUNK - 1 and j >= n_pieces - NACC),
            )

    def drain_b():
        # psum_b completes with the last load_chunk; copy out early so the
        # endgame only waits on the a-stream (copies on ScalarE: close to PSUM)
        for k in range(NACC):
            res = single.tile([P, 256], F32, name=f"resb_{k}", tag="resb", bufs=2)
            nc.scalar.copy(out=res, in_=psum_b[k])
            nc.sync.dma_start(
                out=out[:, NACC * AW + k * 256:NACC * AW + (k + 1) * 256], in_=res)
        nc.sync.dma_start(out=out[:, NACC * (AW + 256):N_OUT], in_=l1pacc)

    # software pipeline: load chunk c while running the stencil on chunk c-1
    for c in range(NCHUNK + 1):
        if c < NCHUNK:
            load_chunk(c)
            if c == NCHUNK - 1:
                drain_b()
        if c >= 1:
            stencil_chunk(c - 1)

    for k in range(NACC):
        res = single.tile([P, AW], F32, name=f"resa_{k}", tag="resa", bufs=2)
        nc.scalar.copy(out=res, in_=psum_a[k])
        nc.sync.dma_start(out=out[:, k * AW:(k + 1) * AW], in_=res)


_CACHED = {}


def _build():
    if "nc" in _CACHED:
        return _CACHED["nc"]
    nc = bacc.Bacc(
        "TRN2",
        target_bir_lowering=False,
        debug=False,
        num_devices=NCORES,
    )
    yp = nc.dram_tensor("y_pred", [IPC, H, W], F32, kind="ExternalInput").ap()
    yt = nc.dram_tensor("y_true", [IPC, H, W], F32, kind="ExternalInput").ap()
    out = nc.dram_tensor("out", [P, N_OUT], F32, kind="ExternalOutput").ap()
    with tile.TileContext(nc) as tc:
        with ExitStack() as ctx:
            _kernel_body(ctx, tc, yp, yt, out)
    nc.compile()
    _CACHED["nc"] = nc
    return nc


def _host_reduce(outs):
    """Assemble the scalar loss from the 8 per-core [P, N_OUT] partial tensors."""
    total = np.float64(0.0)
    idx = np.arange(P)
    for o in outs:
        o = np.asarray(o, dtype=np.float64)
        a = o[:, 0:NACC * AW].reshape(P, NACC, AW).sum(axis=1)
        bq = o[:, NACC * AW:NACC * (AW + 256)].reshape(P, NACC, 256).sum(axis=1)
        l1 = o[:, NACC * (AW + 256):NACC * (AW + 256) + NCHUNK]
        sum_ur = a[idx, idx].sum()          # sum U * relu(x-.5)
        sum_us = a[idx, 128 + idx].sum()    # sum U * sign(x-.5)
        sum_u = a[:, 256].sum()             # sum U
        sum_ylp = bq[idx, idx].sum()        # sum yt * ln(x)
        sum_yl1p = bq[idx, 128 + idx].sum() # sum yt * ln(1-x)
        sum_l1p = l1.sum()                  # sum ln(1-x)
        # thred = R + 0.25*s + 0.25
        total += (sum_ur + 0.25 * sum_us + 0.25 * sum_u) \
            - sum_ylp - sum_l1p + sum_yl1p
    return np.float32(total / (B * H * W))


def kernel(y_true, y_pred):
    y_true = np.ascontiguousarray(np.asarray(y_true, dtype=np.float32)).reshape(B, H, W)
    y_pred = np.ascontiguousarray(np.asarray(y_pred, dtype=np.float32)).reshape(B, H, W)

    nc = _build()
    in_maps = []
    for r in range(NCORES):
        in_maps.append({
            "y_pred": np.ascontiguousarray(y_pred[r * IPC:(r + 1) * IPC]),
            "y_true": np.ascontiguousarray(y_true[r * IPC:(r + 1) * IPC]),
        })
    res = run_bass_kernel_spmd(nc, in_maps, core_ids=list(range(NCORES)))
    outs = [res.results[r]["out"] for r in range(NCORES)]
    return _host_reduce(outs)


# revision 18
# speedup vs baseline: 1.0972x; 1.0701x over previous
"""Trainium2 Bass kernel for nn_ConsistencyLoss (BCE + dilated-stencil consistency loss).

loss = mean( unfolded_weights * thred + bce )
  bce      = -(y_true*max(log(y_pred),-100) + (1-y_true)*max(log1p(-y_pred),-100))
  unfolded = max over 8 dilated (DIL=2) neighbors nb of |y_pred - nb|, zero-padded
  thred    = y_pred * (y_pred >= 0.5)

Strategy (8 NeuronCores, data-parallel over batch, 2 images/core):
  - Chunk tiles [128, 4096] = 2 bands x 2 images, blocks [i0b0|i0b1|i1b0|i1b1];
    one 4D-AP DMA per tensor per chunk (casting loads for bf16 copies).
  - unfolded = max(c - nmin, nmax - c); nmax/nmin separable over the dilated
    3x3 window INCLUDING the center (|c-c| = 0 never changes the max).
  - Vertical (partition) shifts via SBUF->SBUF DMA; horizontal shifts via
    free-dim slices of zero-padded tiles. Stencil in bf16 on DVE (2x mode).
  - BCE logs + relu(x-.5) + sign(x-.5) on ScalarE: ln(x + FLT_MIN)
    reproduces torch's -100 clamp for uniform inputs (only x == 0 clamps).
    thred = R + 0.25*s + 0.25 with R = relu(x-.5), s = sign(x-.5).
  - Product-sums via TensorE diagonal matmuls accumulated in PSUM:
    a-stream rhs pieces [R_j | s_j | 1] (FD=257, the ones column yields
    sum(U) for free), b-stream [lp_j | l1p_j]; 4 round-robin accumulators
    per stream; sum(l1p) rides the ACT accum_out. Host assembles the scalar.
"""

from contextlib import ExitStack

import numpy as np

import concourse.bacc as bacc
import concourse.tile as tile
from concourse import mybir
from concourse.bass_utils import run_bass_kernel_spmd

F32 = mybir.dt.float32
BF16 = mybir.dt.bfloat16
OP = mybir.AluOpType
AT = mybir.ActivationFunctionType

B, H, W = 16, 1024, 1024
NCORES = 8
IPC = B // NCORES          # images per core = 2
P = 128
NB = 2                     # bands per image per chunk tile
NBLK = IPC * NB            # 4 column blocks per chunk tile
NCHUNK = H // (P * NB)     # 4 chunk iterations
FW = NBLK * W              # 4096
BW = W + 4                 # padded block width
DIL = 2
TINY = 1.18e-38            # min normal fp32; ln(x+TINY) == ln(x) for x >= 2^-24

NACC = 4                   # round-robin PSUM accumulators per stream
RSTR = 260                 # rhs piece stride (els) in the [R|s|1] tile (8B-aligned)
AW = 257                   # a-stream rhs width: [R(128) | s(128) | ones(1)]
N_OUT = NACC * AW + NACC * 256 + NCHUNK


def _kernel_body(ctx, tc, yp, yt, out):
    nc = tc.nc

    xpool = ctx.enter_context(tc.tile_pool(name="xpool", bufs=2))
    xbpool = ctx.enter_context(tc.tile_pool(name="xbpool", bufs=3))
    ytpool = ctx.enter_context(tc.tile_pool(name="ytpool", bufs=1))
    fpool = ctx.enter_context(tc.tile_pool(name="fpool", bufs=2))    # lpl1p / rs1
    shpool = ctx.enter_context(tc.tile_pool(name="shpool", bufs=1))  # xu/xd
    vpool = ctx.enter_context(tc.tile_pool(name="vpool", bufs=1))    # padded vmax/vmin
    spool = ctx.enter_context(tc.tile_pool(name="spool", bufs=1))    # stencil temps
    upool = ctx.enter_context(tc.tile_pool(name="upool", bufs=1))
    single = ctx.enter_context(tc.tile_pool(name="single", bufs=1))
    psum = ctx.enter_context(tc.tile_pool(name="psum", bufs=1, space="PSUM"))

    l1pacc = single.tile([P, NCHUNK], F32)
    psum_a = [psum.tile([P, AW], F32, name=f"psum_a{k}") for k in range(NACC)]
    psum_b = [psum.tile([P, 256], F32, name=f"psum_b{k}") for k in range(NACC)]

    bias_tiny = single.tile([P, 1], F32)
    nc.gpsimd.memset(bias_tiny, TINY)
    bias_one = single.tile([P, 1], F32)
    nc.gpsimd.memset(bias_one, 1.0)
    bias_neghalf = single.tile([P, 1], F32)
    nc.gpsimd.memset(bias_neghalf, -0.5)

    zrow = single.tile([DIL, W], BF16)
    nc.vector.memset(zrow, 0.0)

    xb_tiles = {}
    rs_tiles = {}

    n_pieces = FW // P  # 32 lhsT pieces per chunk per stream

    def chunk_src(t, c, img):
        """[NB*P, W] DRAM rows of chunk c, image img -> [P, band, w] 3D AP."""
        return t[img, c * NB * P:(c + 1) * NB * P, :].rearrange(
            "(s p) w -> p s w", p=P)

    def load_chunk(c):
        x = xpool.tile([P, FW], F32, name=f"x_{c}", tag="x")
        xb = xbpool.tile([P, FW], BF16, name=f"xb_{c}", tag="xb")
        ytb = ytpool.tile([P, FW], BF16, name=f"ytb_{c}", tag="ytb")
        for img in range(IPC):
            h0 = img * NB * W
            o3 = lambda t: t[:, h0:h0 + NB * W].rearrange("p (s w) -> p s w", s=NB)
            nc.sync.dma_start(out=o3(x), in_=chunk_src(yp, c, img))
            nc.gpsimd.dma_start(out=o3(ytb), in_=chunk_src(yt, c, img))
        # bf16 stencil copy on ACT (first in the chunk's ACT queue: the
        # vertical-shift DMAs and the whole DVE chain hang off it)
        nc.scalar.copy(out=xb, in_=x)
        xb_tiles[c] = xb

        # [lp|l1p] interleaved at 128 cols: piece j occupies cols [256j, 256j+256)
        lpl1p = fpool.tile([P, 2 * FW], BF16, name=f"lpl1p_{c}", tag="lpl1p")
        lp4 = lpl1p.rearrange("p (j t w) -> p j t w", t=2, w=P)
        nc.scalar.activation(lp4[:, :, 0, :], x, AT.Ln, bias=bias_tiny, scale=1.0)
        nc.scalar.activation(
            lp4[:, :, 1, :], x, AT.Ln, bias=bias_one, scale=-1.0,
            accum_out=l1pacc[:, c:c + 1],
        )

        # [R|s|1] pieces with stride RSTR; R, s on ACT; ones via memset
        rs1 = fpool.tile([P, n_pieces * RSTR], BF16, name=f"rs1_{c}", tag="rs1")
        rs4 = rs1.rearrange("p (j w) -> p j w", j=n_pieces)
        nc.scalar.activation(rs4[:, :, 0:P], x, AT.Relu, bias=bias_neghalf, scale=1.0)
        nc.scalar.activation(rs4[:, :, P:2 * P], x, AT.Sign, bias=bias_neghalf, scale=1.0)
        nc.gpsimd.memset(rs4[:, :, 2 * P:2 * P + 1], 1.0)
        rs_tiles[c] = rs1

        # BCE product-sums: psum_b[m, :] += sum_k ytb[k, 128j+m] * [lp|l1p](j)[k, :]
        for j in range(n_pieces):
            nc.tensor.matmul(
                psum_b[j % NACC],
                ytb[:, j * P:(j + 1) * P],
                lpl1p[:, j * 256:(j + 1) * 256],
                start=(c == 0 and j < NACC),
                stop=(c == NCHUNK - 1 and j >= n_pieces - NACC),
            )

    def stencil_chunk(c):
        xbc = xb_tiles[c]

        # vertical +-2 partition shifts; per-block halo fixups
        xu = shpool.tile([P, FW], BF16, name=f"xu_{c}", tag="xu")
        xd = shpool.tile([P, FW], BF16, name=f"xd_{c}", tag="xd")
        nc.sync.dma_start(out=xu[0:P - DIL, :], in_=xbc[DIL:P, :])
        nc.sync.dma_start(out=xd[DIL:P, :], in_=xbc[0:P - DIL, :])
        for img in range(IPC):
            for s in range(NB):
                q = img * NB + s
                c0, c1 = q * W, (q + 1) * W
                # bottom halo of block q: first rows of the next band down
                if s + 1 < NB:
                    n0 = (img * NB + s + 1) * W
                    nc.sync.dma_start(
                        out=xu[P - DIL:P, c0:c1], in_=xbc[0:DIL, n0:n0 + W])
                elif c + 1 < NCHUNK:
                    n0 = (img * NB) * W
                    nc.sync.dma_start(
                        out=xu[P - DIL:P, c0:c1],
                        in_=xb_tiles[c + 1][0:DIL, n0:n0 + W])
                else:
                    nc.sync.dma_start(out=xu[P - DIL:P, c0:c1], in_=zrow)
                # top halo of block q: last rows of the previous band up
                if s > 0:
                    n0 = (img * NB + s - 1) * W
                    nc.sync.dma_start(
                        out=xd[0:DIL, c0:c1], in_=xbc[P - DIL:P, n0:n0 + W])
                elif c > 0:
                    n0 = (img * NB + NB - 1) * W
                    nc.sync.dma_start(
                        out=xd[0:DIL, c0:c1],
                        in_=xb_tiles[c - 1][P - DIL:P, n0:n0 + W])
                else:
                    nc.sync.dma_start(out=xd[0:DIL, c0:c1], in_=zrow)

        # vertical 3-max / 3-min into zero-padded tiles
        vmax = vpool.tile([P, NBLK * BW], BF16, name=f"vmax_{c}", tag="vmax")
        vmin = vpool.tile([P, NBLK * BW], BF16, name=f"vmin_{c}", tag="vmin")
        for v in (vmax, vmin):
            for q in range(NBLK):
                nc.gpsimd.memset(v[:, q * BW:q * BW + 2], 0.0)
                nc.gpsimd.memset(v[:, q * BW + BW - 2:(q + 1) * BW], 0.0)
        vmax3 = vmax.rearrange("p (q w) -> p q w", q=NBLK)
        vmin3 = vmin.rearrange("p (q w) -> p q w", q=NBLK)

        def b3(t):
            return t.rearrange("p (q w) -> p q w", q=NBLK)

        va = spool.tile([P, FW], BF16, name=f"va_{c}", tag="g1")
        nc.vector.tensor_tensor(out=va, in0=xu, in1=xd, op=OP.max)
        nc.vector.tensor_tensor(
            out=vmax3[:, :, 2:2 + W], in0=b3(va), in1=b3(xbc), op=OP.max)
        vb = spool.tile([P, FW], BF16, name=f"vb_{c}", tag="g2")
        nc.vector.tensor_tensor(out=vb, in0=xu, in1=xd, op=OP.min)
        nc.vector.tensor_tensor(
            out=vmin3[:, :, 2:2 + W], in0=b3(vb), in1=b3(xbc), op=OP.min)

        # horizontal dilated 3-max / 3-min
        nxa = spool.tile([P, FW], BF16, name=f"nxa_{c}", tag="g1")
        nc.vector.tensor_tensor(
            out=b3(nxa), in0=vmax3[:, :, 0:W], in1=vmax3[:, :, 4:4 + W], op=OP.max)
        nx = spool.tile([P, FW], BF16, name=f"nx_{c}", tag="g3")
        nc.vector.tensor_tensor(
            out=b3(nx), in0=b3(nxa), in1=vmax3[:, :, 2:2 + W], op=OP.max)
        nma = spool.tile([P, FW], BF16, name=f"nma_{c}", tag="g2")
        nc.vector.tensor_tensor(
            out=b3(nma), in0=vmin3[:, :, 0:W], in1=vmin3[:, :, 4:4 + W], op=OP.min)
        nm = spool.tile([P, FW], BF16, name=f"nm_{c}", tag="g4")
        nc.vector.tensor_tensor(
            out=b3(nm), in0=b3(nma), in1=vmin3[:, :, 2:2 + W], op=OP.min)

        # unfolded = max(xb - nmin, nmax - xb)
        u1 = spool.tile([P, FW], BF16, name=f"u1_{c}", tag="g1")
        nc.vector.tensor_tensor(out=u1, in0=xbc, in1=nm, op=OP.subtract)
        u2 = spool.tile([P, FW], BF16, name=f"u2_{c}", tag="g2")
        nc.vector.tensor_tensor(out=u2, in0=nx, in1=xbc, op=OP.subtract)
        u = upool.tile([P, FW], BF16, name=f"u_{c}", tag="u")
        nc.vector.tensor_tensor(out=u, in0=u1, in1=u2, op=OP.max)

        # psum_a[m, :] += sum_k u[k, 128j+m] * [R|s|1](j)[k, :]
        rsc = rs_tiles[c]
        for j in range(n_pieces):
            nc.tensor.matmul(
                psum_a[j % NACC],
                u[:, j * P:(j + 1) * P],
                rsc[:, j * RSTR:j * RSTR + AW],
                start=(c == 0 and j < NACC),
                stop=(c == NCHUNK - 1 and j >= n_pieces - NACC),
            )

    def drain_b():
        # psum_b completes with the last load_chunk; copy out early so the
        # endgame only waits on the a-stream (copies on ScalarE: close to PSUM)
        for k in range(NACC):
            res = single.tile([P, 256], F32, name=f"resb_{k}", tag="resb", bufs=2)
            nc.scalar.copy(out=res, in_=psum_b[k])
            nc.sync.dma_start(
                out=out[:, NACC * AW + k * 256:NACC * AW + (k + 1) * 256], in_=res)
        nc.sync.dma_start(out=out[:, NACC * (AW + 256):N_OUT], in_=l1pacc)

    # software pipeline: load chunk c while running the stencil on chunk c-1
    for c in range(NCHUNK + 1):
        if c < NCHUNK:
            load_chunk(c)
            if c == NCHUNK - 1:
                drain_b()
        if c >= 1:
            stencil_chunk(c - 1)

    for k in range(NACC):
        res = single.tile([P, AW], F32, name=f"resa_{k}", tag="resa", bufs=2)
        nc.scalar.copy(out=res, in_=psum_a[k])
        nc.sync.dma_start(out=out[:, k * AW:(k + 1) * AW], in_=res)


_CACHED = {}


def _build():
    if "nc" in _CACHED:
        return _CACHED["nc"]
    nc = bacc.Bacc(
        "TRN2",
        target_bir_lowering=False,
        debug=False,
        num_devices=NCORES,
    )
    yp = nc.dram_tensor("y_pred", [IPC, H, W], F32, kind="ExternalInput").ap()
    yt = nc.dram_tensor("y_true", [IPC, H, W], F32, kind="ExternalInput").ap()
    out = nc.dram_tensor("out", [P, N_OUT], F32, kind="ExternalOutput").ap()
    with tile.TileContext(nc) as tc:
        with ExitStack() as ctx:
            _kernel_body(ctx, tc, yp, yt, out)
    nc.compile()
    _CACHED["nc"] = nc
    return nc


def _host_reduce(outs):
    """Assemble the scalar loss from the 8 per-core [P, N_OUT] partial tensors."""
    total = np.float64(0.0)
    idx = np.arange(P)
    for o in outs:
        o = np.asarray(o, dtype=np.float64)
        a = o[:, 0:NACC * AW].reshape(P, NACC, AW).sum(axis=1)
        bq = o[:, NACC * AW:NACC * (AW + 256)].reshape(P, NACC, 256).sum(axis=1)
        l1 = o[:, NACC * (AW + 256):NACC * (AW + 256) + NCHUNK]
        sum_ur = a[idx, idx].sum()          # sum U * relu(x-.5)
        sum_us = a[idx, 128 + idx].sum()    # sum U * sign(x-.5)
        sum_u = a[:, 256].sum()             # sum U
        sum_ylp = bq[idx, idx].sum()        # sum yt * ln(x)
        sum_yl1p = bq[idx, 128 + idx].sum() # sum yt * ln(1-x)
        sum_l1p = l1.sum()                  # sum ln(1-x)
        # thred = R + 0.25*s + 0.25
        total += (sum_ur + 0.25 * sum_us + 0.25 * sum_u) \
            - sum_ylp - sum_l1p + sum_yl1p
    return np.float32(total / (B * H * W))


def kernel(y_true, y_pred):
    y_true = np.ascontiguousarray(np.asarray(y_true, dtype=np.float32)).reshape(B, H, W)
    y_pred = np.ascontiguousarray(np.asarray(y_pred, dtype=np.float32)).reshape(B, H, W)

    nc = _build()
    in_maps = []
    for r in range(NCORES):
        in_maps.append({
            "y_pred": np.ascontiguousarray(y_pred[r * IPC:(r + 1) * IPC]),
            "y_true": np.ascontiguousarray(y_true[r * IPC:(r + 1) * IPC]),
        })
    res = run_bass_kernel_spmd(nc, in_maps, core_ids=list(range(NCORES)))
    outs = [res.results[r]["out"] for r in range(NCORES)]
    return _host_reduce(outs)


# revision 20
# speedup vs baseline: 1.1143x; 1.0156x over previous
"""Trainium2 Bass kernel for nn_ConsistencyLoss (BCE + dilated-stencil consistency loss).

loss = mean( unfolded_weights * thred + bce )
  bce      = -(y_true*max(log(y_pred),-100) + (1-y_true)*max(log1p(-y_pred),-100))
  unfolded = max over 8 dilated (DIL=2) neighbors nb of |y_pred - nb|, zero-padded
  thred    = y_pred * (y_pred >= 0.5)

Strategy (8 NeuronCores, data-parallel over batch, 2 images/core):
  - Chunk tiles [128, 4096] = 2 bands x 2 images, blocks [i0b0|i0b1|i1b0|i1b1];
    one 4D-AP DMA per tensor per chunk (casting loads for bf16 copies).
  - unfolded = max(c - nmin, nmax - c); nmax/nmin separable over the dilated
    3x3 window INCLUDING the center (|c-c| = 0 never changes the max).
  - Vertical (partition) shifts via SBUF->SBUF DMA; horizontal shifts via
    free-dim slices of zero-padded tiles. Stencil in bf16 on DVE (2x mode).
  - BCE logs + relu(x-.5) + sign(x-.5) on ScalarE: ln(x + FLT_MIN)
    reproduces torch's -100 clamp for uniform inputs (only x == 0 clamps).
    thred = R + 0.25*s + 0.25 with R = relu(x-.5), s = sign(x-.5).
  - Product-sums via TensorE diagonal matmuls accumulated in PSUM:
    a-stream rhs pieces [R_j | s_j | 1] (FD=257, the ones column yields
    sum(U) for free), b-stream [lp_j | l1p_j]; 4 round-robin accumulators
    per stream; sum(l1p) rides the ACT accum_out. Host assembles the scalar.
"""

from contextlib import ExitStack

import numpy as np

import concourse.bacc as bacc
import concourse.tile as tile
from concourse import mybir
from concourse.bass_utils import run_bass_kernel_spmd

F32 = mybir.dt.float32
BF16 = mybir.dt.bfloat16
OP = mybir.AluOpType
AT = mybir.ActivationFunctionType

B, H, W = 16, 1024, 1024
NCORES = 8
IPC = B // NCORES          # images per core = 2
P = 128
NB = 2                     # bands per image per chunk tile
NBLK = IPC * NB            # 4 column blocks per chunk tile
NCHUNK = H // (P * NB)     # 4 chunk iterations
FW = NBLK * W              # 4096
BW = W + 4                 # padded block width
DIL = 2
TINY = 1.18e-38            # min normal fp32; ln(x+TINY) == ln(x) for x >= 2^-24

NACC = 4                   # round-robin PSUM accumulators per stream
RSTR = 260                 # rhs piece stride (els) in the [R|s|1] tile (8B-aligned)
AW = 257                   # a-stream rhs width: [R(128) | s(128) | ones(1)]
N_OUT = NACC * AW + NACC * 256 + NCHUNK


def _kernel_body(ctx, tc, yp, yt, out):
    nc = tc.nc

    xpool = ctx.enter_context(tc.tile_pool(name="xpool", bufs=2))
    xbpool = ctx.enter_context(tc.tile_pool(name="xbpool", bufs=3))
    ytpool = ctx.enter_context(tc.tile_pool(name="ytpool", bufs=2))
    fpool = ctx.enter_context(tc.tile_pool(name="fpool", bufs=2))    # lpl1p / rs1
    shpool = ctx.enter_context(tc.tile_pool(name="shpool", bufs=1))  # xu/xd
    vpool = ctx.enter_context(tc.tile_pool(name="vpool", bufs=1))    # padded vmax/vmin
    spool = ctx.enter_context(tc.tile_pool(name="spool", bufs=1))    # stencil temps
    upool = ctx.enter_context(tc.tile_pool(name="upool", bufs=1))
    single = ctx.enter_context(tc.tile_pool(name="single", bufs=1))
    psum = ctx.enter_context(tc.tile_pool(name="psum", bufs=1, space="PSUM"))

    l1pacc = single.tile([P, NCHUNK], F32)
    psum_a = [psum.tile([P, AW], F32, name=f"psum_a{k}") for k in range(NACC)]
    psum_b = [psum.tile([P, 256], F32, name=f"psum_b{k}") for k in range(NACC)]

    bias_tiny = single.tile([P, 1], F32)
    nc.gpsimd.memset(bias_tiny, TINY)
    bias_one = single.tile([P, 1], F32)
    nc.gpsimd.memset(bias_one, 1.0)
    bias_neghalf = single.tile([P, 1], F32)
    nc.gpsimd.memset(bias_neghalf, -0.5)

    zrow = single.tile([DIL, W], BF16)
    nc.vector.memset(zrow, 0.0)

    xb_tiles = {}
    rs_tiles = {}

    n_pieces = FW // P  # 32 lhsT pieces per chunk per stream

    def chunk_src(t, c, img):
        """[NB*P, W] DRAM rows of chunk c, image img -> [P, band, w] 3D AP."""
        return t[img, c * NB * P:(c + 1) * NB * P, :].rearrange(
            "(s p) w -> p s w", p=P)

    x_tiles = {}
    yt_tiles = {}

    def load_chunk(c):
        """x/ytb loads + the ACT xb copy — issued one iteration ahead of the
        field passes so the vertical-shift DMAs (and the whole DVE chain)
        never wait behind a chunk's ln/relu/sign ACT queue."""
        x = xpool.tile([P, FW], F32, name=f"x_{c}", tag="x")
        xb = xbpool.tile([P, FW], BF16, name=f"xb_{c}", tag="xb")
        ytb = ytpool.tile([P, FW], BF16, name=f"ytb_{c}", tag="ytb")
        for img in range(IPC):
            h0 = img * NB * W
            o3 = lambda t: t[:, h0:h0 + NB * W].rearrange("p (s w) -> p s w", s=NB)
            nc.sync.dma_start(out=o3(x), in_=chunk_src(yp, c, img))
            nc.gpsimd.dma_start(out=o3(ytb), in_=chunk_src(yt, c, img))
        nc.scalar.copy(out=xb, in_=x)
        xb_tiles[c] = xb
        x_tiles[c] = x
        yt_tiles[c] = ytb

    def field_chunk(c):
        x = x_tiles[c]
        ytb = yt_tiles[c]

        # [lp|l1p] interleaved at 128 cols: piece j occupies cols [256j, 256j+256)
        lpl1p = fpool.tile([P, 2 * FW], BF16, name=f"lpl1p_{c}", tag="lpl1p", bufs=1)
        lp4 = lpl1p.rearrange("p (j t w) -> p j t w", t=2, w=P)
        nc.scalar.activation(lp4[:, :, 0, :], x, AT.Ln, bias=bias_tiny, scale=1.0)
        nc.scalar.activation(
            lp4[:, :, 1, :], x, AT.Ln, bias=bias_one, scale=-1.0,
            accum_out=l1pacc[:, c:c + 1],
        )

        # [R|s|1] pieces with stride RSTR; R, s on ACT; ones via memset
        rs1 = fpool.tile([P, n_pieces * RSTR], BF16, name=f"rs1_{c}", tag="rs1")
        rs4 = rs1.rearrange("p (j w) -> p j w", j=n_pieces)
        nc.scalar.activation(rs4[:, :, 0:P], x, AT.Relu, bias=bias_neghalf, scale=1.0)
        nc.scalar.activation(rs4[:, :, P:2 * P], x, AT.Sign, bias=bias_neghalf, scale=1.0)
        nc.gpsimd.memset(rs4[:, :, 2 * P:2 * P + 1], 1.0)
        rs_tiles[c] = rs1

        # BCE product-sums: psum_b[m, :] += sum_k ytb[k, 128j+m] * [lp|l1p](j)[k, :]
        for j in range(n_pieces):
            nc.tensor.matmul(
                psum_b[j % NACC],
                ytb[:, j * P:(j + 1) * P],
                lpl1p[:, j * 256:(j + 1) * 256],
                start=(c == 0 and j < NACC),
                stop=(c == NCHUNK - 1 and j >= n_pieces - NACC),
            )

    def stencil_chunk(c):
        xbc = xb_tiles[c]

        # vertical +-2 partition shifts; per-block halo fixups
        xu = shpool.tile([P, FW], BF16, name=f"xu_{c}", tag="xu")
        xd = shpool.tile([P, FW], BF16, name=f"xd_{c}", tag="xd")
        nc.sync.dma_start(out=xu[0:P - DIL, :], in_=xbc[DIL:P, :])
        nc.sync.dma_start(out=xd[DIL:P, :], in_=xbc[0:P - DIL, :])
        for img in range(IPC):
            for s in range(NB):
                q = img * NB + s
                c0, c1 = q * W, (q + 1) * W
                # bottom halo of block q: first rows of the next band down
                if s + 1 < NB:
                    n0 = (img * NB + s + 1) * W
                    nc.sync.dma_start(
                        out=xu[P - DIL:P, c0:c1], in_=xbc[0:DIL, n0:n0 + W])
                elif c + 1 < NCHUNK:
                    n0 = (img * NB) * W
                    nc.sync.dma_start(
                        out=xu[P - DIL:P, c0:c1],
                        in_=xb_tiles[c + 1][0:DIL, n0:n0 + W])
                else:
                    nc.sync.dma_start(out=xu[P - DIL:P, c0:c1], in_=zrow)
                # top halo of block q: last rows of the previous band up
                if s > 0:
                    n0 = (img * NB + s - 1) * W
                    nc.sync.dma_start(
                        out=xd[0:DIL, c0:c1], in_=xbc[P - DIL:P, n0:n0 + W])
                elif c > 0:
                    n0 = (img * NB + NB - 1) * W
                    nc.sync.dma_start(
                        out=xd[0:DIL, c0:c1],
                        in_=xb_tiles[c - 1][P - DIL:P, n0:n0 + W])
                else:
                    nc.sync.dma_start(out=xd[0:DIL, c0:c1], in_=zrow)

        # vertical 3-max / 3-min into zero-padded tiles
        vmax = vpool.tile([P, NBLK * BW], BF16, name=f"vmax_{c}", tag="vmax")
        vmin = vpool.tile([P, NBLK * BW], BF16, name=f"vmin_{c}", tag="vmin")
        for v in (vmax, vmin):
            for q in range(NBLK):
                nc.gpsimd.memset(v[:, q * BW:q * BW + 2], 0.0)
                nc.gpsimd.memset(v[:, q * BW + BW - 2:(q + 1) * BW], 0.0)
        vmax3 = vmax.rearrange("p (q w) -> p q w", q=NBLK)
        vmin3 = vmin.rearrange("p (q w) -> p q w", q=NBLK)

        def b3(t):
            return t.rearrange("p (q w) -> p q w", q=NBLK)

        va = spool.tile([P, FW], BF16, name=f"va_{c}", tag="g1")
        nc.vector.tensor_tensor(out=va, in0=xu, in1=xd, op=OP.max)
        nc.vector.tensor_tensor(
            out=vmax3[:, :, 2:2 + W], in0=b3(va), in1=b3(xbc), op=OP.max)
        vb = spool.tile([P, FW], BF16, name=f"vb_{c}", tag="g2")
        nc.vector.tensor_tensor(out=vb, in0=xu, in1=xd, op=OP.min)
        nc.vector.tensor_tensor(
            out=vmin3[:, :, 2:2 + W], in0=b3(vb), in1=b3(xbc), op=OP.min)

        # horizontal dilated 3-max / 3-min
        nxa = spool.tile([P, FW], BF16, name=f"nxa_{c}", tag="g1")
        nc.vector.tensor_tensor(
            out=b3(nxa), in0=vmax3[:, :, 0:W], in1=vmax3[:, :, 4:4 + W], op=OP.max)
        nx = spool.tile([P, FW], BF16, name=f"nx_{c}", tag="g3")
        nc.vector.tensor_tensor(
            out=b3(nx), in0=b3(nxa), in1=vmax3[:, :, 2:2 + W], op=OP.max)
        nma = spool.tile([P, FW], BF16, name=f"nma_{c}", tag="g2")
        nc.vector.tensor_tensor(
            out=b3(nma), in0=vmin3[:, :, 0:W], in1=vmin3[:, :, 4:4 + W], op=OP.min)
        nm = spool.tile([P, FW], BF16, name=f"nm_{c}", tag="g4")
        nc.vector.tensor_tensor(
            out=b3(nm), in0=b3(nma), in1=vmin3[:, :, 2:2 + W], op=OP.min)

        # unfolded = max(xb - nmin, nmax - xb)
        u1 = spool.tile([P, FW], BF16, name=f"u1_{c}", tag="g1")
        nc.vector.tensor_tensor(out=u1, in0=xbc, in1=nm, op=OP.subtract)
        u2 = spool.tile([P, FW], BF16, name=f"u2_{c}", tag="g2")
        nc.vector.tensor_tensor(out=u2, in0=nx, in1=xbc, op=OP.subtract)
        u = upool.tile([P, FW], BF16, name=f"u_{c}", tag="u")
        nc.vector.tensor_tensor(out=u, in0=u1, in1=u2, op=OP.max)

        # psum_a[m, :] += sum_k u[k, 128j+m] * [R|s|1](j)[k, :]
        rsc = rs_tiles[c]
        for j in range(n_pieces):
            nc.tensor.matmul(
                psum_a[j % NACC],
                u[:, j * P:(j + 1) * P],
                rsc[:, j * RSTR:j * RSTR + AW],
                start=(c == 0 and j < NACC),
                stop=(c == NCHUNK - 1 and j >= n_pieces - NACC),
            )

    def drain_b():
        # psum_b completes with the last load_chunk; copy out early so the
        # endgame only waits on the a-stream (copies on ScalarE: close to PSUM)
        for k in range(NACC):
            res = single.tile([P, 256], F32, name=f"resb_{k}", tag="resb", bufs=2)
            nc.scalar.copy(out=res, in_=psum_b[k])
            nc.sync.dma_start(
                out=out[:, NACC * AW + k * 256:NACC * AW + (k + 1) * 256], in_=res)
        nc.sync.dma_start(out=out[:, NACC * (AW + 256):N_OUT], in_=l1pacc)

    # software pipeline: loads lead the field/stencil passes by one chunk
    for c in range(NCHUNK + 1):
        if c < NCHUNK:
            load_chunk(c)
        if c >= 1:
            field_chunk(c - 1)
            if c == NCHUNK:
                drain_b()
            stencil_chunk(c - 1)

    for k in range(NACC):
        res = single.tile([P, AW], F32, name=f"resa_{k}", tag="resa", bufs=2)
        nc.scalar.copy(out=res, in_=psum_a[k])
        nc.sync.dma_start(out=out[:, k * AW:(k + 1) * AW], in_=res)


_CACHED = {}


def _build():
    if "nc" in _CACHED:
        return _CACHED["nc"]
    nc = bacc.Bacc(
        "TRN2",
        target_bir_lowering=False,
        debug=False,
        num_devices=NCORES,
    )
    yp = nc.dram_tensor("y_pred", [IPC, H, W], F32, kind="ExternalInput").ap()
    yt = nc.dram_tensor("y_true", [IPC, H, W], F32, kind="ExternalInput").ap()
    out = nc.dram_tensor("out", [P, N_OUT], F32, kind="ExternalOutput").ap()
    with tile.TileContext(nc) as tc:
        with ExitStack() as ctx:
            _kernel_body(ctx, tc, yp, yt, out)
    nc.compile()
    _CACHED["nc"] = nc
    return nc


def _host_reduce(outs):
    """Assemble the scalar loss from the 8 per-core [P, N_OUT] partial tensors."""
    total = np.float64(0.0)
    idx = np.arange(P)
    for o in outs:
        o = np.asarray(o, dtype=np.float64)
        a = o[:, 0:NACC * AW].reshape(P, NACC, AW).sum(axis=1)
        bq = o[:, NACC * AW:NACC * (AW + 256)].reshape(P, NACC, 256).sum(axis=1)
        l1 = o[:, NACC * (AW + 256):NACC * (AW + 256) + NCHUNK]
        sum_ur = a[idx, idx].sum()          # sum U * relu(x-.5)
        sum_us = a[idx, 128 + idx].sum()    # sum U * sign(x-.5)
        sum_u = a[:, 256].sum()             # sum U
        sum_ylp = bq[idx, idx].sum()        # sum yt * ln(x)
        sum_yl1p = bq[idx, 128 + idx].sum() # sum yt * ln(1-x)
        sum_l1p = l1.sum()                  # sum ln(1-x)
        # thred = R + 0.25*s + 0.25
        total += (sum_ur + 0.25 * sum_us + 0.25 * sum_u) \
            - sum_ylp - sum_l1p + sum_yl1p
    return np.float32(total / (B * H * W))


def kernel(y_true, y_pred):
    y_true = np.ascontiguousarray(np.asarray(y_true, dtype=np.float32)).reshape(B, H, W)
    y_pred = np.ascontiguousarray(np.asarray(y_pred, dtype=np.float32)).reshape(B, H, W)

    nc = _build()
    in_maps = []
    for r in range(NCORES):
        in_maps.append({
            "y_pred": np.ascontiguousarray(y_pred[r * IPC:(r + 1) * IPC]),
            "y_true": np.ascontiguousarray(y_true[r * IPC:(r + 1) * IPC]),
        })
    res = run_bass_kernel_spmd(nc, in_maps, core_ids=list(range(NCORES)))
    outs = [res.results[r]["out"] for r in range(NCORES)]
    return _host_reduce(outs)
